# revision 45
# baseline (speedup 1.0000x reference)
"""Trainium2 Bass kernel for a 2-layer multi-head GAT (nn_MultiHeadGATLayer).

Architecture recap (hardcoded, matches the reference):
  N=16384 nodes, D=512 feats, E=540672 edges (32 random in-edges/node + self loop),
  layer 1: 8 heads x 64 dims with per-head attention + elu, concat;
  layer 2: single 512-dim GAT head over the concat + elu; residual with input.

Distribution: nodes are sharded across 8 NeuronCores (destination sharding).
Each core computes z = x @ W for its node shard; shards are AllGathered into
a per-core z-table in HBM (4 contiguous chunks, chunk-major table layout, so
the collective pipelines behind the producing z tiles). Each core then runs
the edge phase for its own destination nodes: SWDGE indirect-DMA gathers of
z[src] rows, round-robined over all 4 qPoolDynamic queues (each queue is
served by a different Q7 core pair, ~2x desc-gen overlap). Layer 1 rows are
bf16 [512 z | 8 es | pad] (1280B) and the attention weighting is a DVE
multiply in 2x mode (duplicated attention pairs) + TensorE identity-matmul
accumulation. Layer 2 rows are fp8e4 [512 z | 1 bf16 es | pad] (768B) and
the attention weighting is done on TensorE directly: diag(att) fp8
stationaries (DG[p,k,j] = ident[p,j]*att[p,k]) with DoubleRow fp8 matmuls
contracting two edge slots per instruction - no per-edge DVE multiply at
all. Softmax skips max-subtraction (logits bounded by construction,
self-loops keep denominators positive; padding slots point at a dummy row
whose embedded es = -3e38 so exp()=0). The z2 tiles and the layer-2
AllGather chunks are interleaved into layer-1's edge phase so only AG1 is
exposed. elu(x) is computed as max(x, exp(-relu(-x))-1): two ScalarE ops +
one DVE op.

Host side does layout only: degree-sorted node permutation, padded gather
index construction (int16, 16-partition wrap, shared by both layers),
weight reshapes and the small va = W @ A logit-weight products.
"""
import os
import sys

sys.path.insert(0, "/opt/trn_rl_repo")

import numpy as np
import ml_dtypes

import concourse.bacc as bacc
import concourse.mybir as mybir
from concourse.tile import TileContext
from concourse.bass_utils import run_bass_kernel_spmd
from concourse.library_config import mlp

F32 = mybir.dt.float32
BF16 = mybir.dt.bfloat16
FP8 = mybir.dt.float8e4
I16 = mybir.dt.int16

B, S, D = 64, 256, 512
H, DO = 8, 64
ALPHA = 0.2
N = B * S
DEG = 32
E = N * DEG + N
NCORES = 8
P = 128
SHN = N // NCORES          # nodes per core (2048)
NT = SHN // P              # node tiles per core (16)
KG = 8                     # slots per gather call
ROWW = 640                 # L1 z-table row (bf16): 512 z + 8 es + pad
ROWB = 768                 # L2 z-table row bytes: 512 fp8 z + 2B bf16 es + pad
DUMMY = N                  # dummy row index for padding slots
CHB = [0, 1024, 1536, 1792, 2048]   # AG chunk row boundaries (per core)
AG_AT = {7: 0, 11: 1, 13: 2, 15: 3}  # z-tile -> AG chunk to fire
NEG = -3.0e38

_cache = {}


def _build_host(src, dst):
    """Host-side layout: permutation, per-core padded gather indices."""
    deg = np.bincount(dst, minlength=N)
    order = np.argsort(-deg, kind="stable")          # nodes by degree desc
    # deal round-robin so all cores see the same degree profile
    core_of = np.empty(N, np.int32)
    pos_of = np.empty(N, np.int32)
    for c in range(NCORES):
        nodes_c = order[c::NCORES]                   # 2048 nodes, deg-sorted desc
        core_of[nodes_c] = c
        pos_of[nodes_c] = np.arange(SHN)
    nodes = [order[c::NCORES] for c in range(NCORES)]
    # non-uniform chunk-major table layout for the pipelined AllGather:
    # big chunks early, small last chunk so the final AG dependency is tiny
    pos64 = pos_of.astype(np.int64)
    ch_of = np.searchsorted(np.array(CHB[1:-1]), pos64, side="right")
    base = np.array([NCORES * b for b in CHB[:-1]])
    csz = np.array([CHB[i + 1] - CHB[i] for i in range(len(CHB) - 1)])
    tabpos = (base[ch_of] + core_of.astype(np.int64) * csz[ch_of]
              + pos64 - np.array(CHB[:-1])[ch_of]).astype(np.int32)

    # in-edges per node: sort edges by dst
    eorder = np.argsort(dst, kind="stable")
    src_s = src[eorder]
    cum = np.zeros(N + 1, np.int64)
    np.cumsum(deg, out=cum[1:])

    # common slot schedule: K_sched[j] = max over cores of tile max degree
    K_sched = np.zeros(NT, np.int64)
    for c in range(NCORES):
        dg = deg[nodes[c]]
        for j in range(NT):
            K_sched[j] = max(K_sched[j], dg[j * P:(j + 1) * P].max())
    K_sched = ((K_sched + KG - 1) // KG) * KG
    totK = int(K_sched.sum())

    # padded source table per core: [NT, K_j, P] slot-major within tile
    idx_cores = []
    for c in range(NCORES):
        blocks = []
        nds = nodes[c]
        for j in range(NT):
            nj = nds[j * P:(j + 1) * P]
            Kj = int(K_sched[j])
            pad = np.full((P, Kj), DUMMY, np.int32)
            for i, n in enumerate(nj):
                d0 = int(deg[n])
                pad[i, :d0] = tabpos[src_s[cum[n]:cum[n] + d0]]
            blocks.append(pad.T.reshape(-1))         # slot-major: [Kj, P]
        flat = np.concatenate(blocks).astype(np.int32)   # [totK*P]
        # int16 wrap layout: idx i -> partition i%16, col i//16 ; replicate 8x
        assert flat.max() <= 32767
        w = flat.reshape(-1, 16).T                   # [16, totK*P/16]
        idx_cores.append(np.tile(w, (8, 1)).astype(np.int16))
    return nodes, K_sched, totK, idx_cores


def _build_program(K_sched, totK):
    nc = bacc.Bacc("TRN2", target_bir_lowering=False, debug=False,
                   num_devices=NCORES, num_swdge_queues=4)
    KT = [int(k) for k in K_sched]
    IDXW = totK * P // 16

    xT_in = nc.dram_tensor("xT", [D, SHN], BF16, kind="ExternalInput")
    x_in = nc.dram_tensor("x", [SHN, D], F32, kind="ExternalInput")
    w1_in = nc.dram_tensor("w1", [D, D], BF16, kind="ExternalInput")      # W1cat
    w2_in = nc.dram_tensor("w2", [D, D], BF16, kind="ExternalInput")      # Wout
    va1_in = nc.dram_tensor("va1", [D, 16], BF16, kind="ExternalInput")   # W1cat@A1
    va2_in = nc.dram_tensor("va2", [D, 2], BF16, kind="ExternalInput")    # Wout@A2
    id_in = nc.dram_tensor("ident", [P, P], BF16, kind="ExternalInput")
    idx1_in = nc.dram_tensor("idx1", [P, IDXW], I16, kind="ExternalInput")
    out = nc.dram_tensor("out", [SHN, D], F32, kind="ExternalOutput")

    zloc1 = nc.dram_tensor("zloc1", [SHN, ROWW], BF16, kind="Internal")
    zloc2 = nc.dram_tensor("zloc2", [SHN, ROWB], FP8, kind="Internal")
    ztab1 = nc.dram_tensor("ztab1", [N + P, ROWW], BF16, kind="Internal",
                           addr_space="Shared")
    ztab2 = nc.dram_tensor("ztab2", [N + P, ROWB], FP8, kind="Internal",
                           addr_space="Shared")

    with TileContext(nc) as tc:
        with tc.tile_pool(name="const", bufs=1) as cpool, \
             tc.tile_pool(name="work", bufs=2) as wpool, \
             tc.tile_pool(name="gat", bufs=5) as gpool, \
             tc.tile_pool(name="wg", bufs=3) as wgpool, \
             tc.tile_pool(name="pz", bufs=2, space="PSUM") as pzpool, \
             tc.tile_pool(name="pe", bufs=2, space="PSUM") as pepool, \
             tc.tile_pool(name="pt", bufs=2, space="PSUM") as ptpool:

            nc.gpsimd.load_library(mlp)

            # ---------- setup: constants, weights ----------
            identb = cpool.tile([P, P], BF16)
            nc.sync.dma_start(identb[:], id_in[:])

            idx1 = cpool.tile([P, IDXW], I16)
            nc.sync.dma_start(idx1[:], idx1_in[:])
            idx2 = idx1

            # weights: bf16 rhs chunks [128, 512] (4 per layer) + va cols
            w1b, w2b, va1, va2 = [], [], [], []
            for cidx in range(4):
                wb = cpool.tile([P, D], BF16, tag=f"w1b{cidx}")
                nc.sync.dma_start(wb[:], w1_in[cidx * P:(cidx + 1) * P, :])
                w1b.append(wb)
                wb2 = cpool.tile([P, D], BF16, tag=f"w2b{cidx}")
                nc.sync.dma_start(wb2[:], w2_in[cidx * P:(cidx + 1) * P, :])
                w2b.append(wb2)
                vb = cpool.tile([P, 16], BF16, tag=f"va1{cidx}")
                nc.sync.dma_start(vb[:], va1_in[cidx * P:(cidx + 1) * P, :])
                va1.append(vb)
                vb2 = cpool.tile([P, 2], BF16, tag=f"va2{cidx}")
                nc.sync.dma_start(vb2[:], va2_in[cidx * P:(cidx + 1) * P, :])
                va2.append(vb2)

            # dummy rows (padding target): zeros except es slots = NEG
            drow1 = cpool.tile([1, ROWW], BF16)
            nc.vector.memset(drow1[:], 0.0)
            nc.vector.memset(drow1[:, D:D + H], NEG)
            nc.sync.dma_start(ztab1[N:N + 1, :], drow1[:])
            drow2 = cpool.tile([1, ROWB], FP8)
            nc.vector.memset(drow2[:], 0.0)
            nc.vector.memset(drow2[:, D:D + 2].bitcast(BF16), NEG)
            nc.sync.dma_start(ztab2[N:N + 1, :], drow2[:])

            identq = cpool.tile([P, P], FP8)
            nc.vector.tensor_copy(identq[:], identb[:])

            ed1 = cpool.tile([P, NT, H], F32)
            ed2 = cpool.tile([P, NT, 1], F32)


            # ---------- phase 1: z1 shard ----------
            xTb = [cpool.tile([P, SHN], BF16, tag=f"xT{c}", name=f"xT{c}") for c in range(4)]
            for cidx in range(4):
                nc.sync.dma_start(xTb[cidx][:], xT_in[cidx * P:(cidx + 1) * P, :])

            def z_tile(nt, lhs_blocks, wb, va, zloc, ed_store, ncols, fp8):
                pa = pzpool.tile([P, D], F32, tag="pzA")
                pb = pzpool.tile([P, 16], F32, tag="pzB", bufs=1)
                for cidx in range(4):
                    lb = lhs_blocks[cidx][:, nt * P:(nt + 1) * P]
                    nc.tensor.matmul(pa[:], lb, wb[cidx][:],
                                     start=(cidx == 0), stop=(cidx == 3))
                for cidx in range(4):
                    lb = lhs_blocks[cidx][:, nt * P:(nt + 1) * P]
                    nc.tensor.matmul(pb[:, 0:2 * ncols], lb, va[cidx][:],
                                     start=(cidx == 0), stop=(cidx == 3))
                if fp8:
                    zrow = wpool.tile([P, ROWB], FP8, tag="zrow2")
                    nc.vector.tensor_copy(zrow[:, 0:D], pa[:])
                    nc.vector.tensor_copy(
                        zrow[:, D:D + 2 * ncols].bitcast(BF16), pb[:, 0:ncols])
                else:
                    zrow = wpool.tile([P, ROWW], BF16, tag="zrow1")
                    nc.vector.tensor_copy(zrow[:, 0:D], pa[:])
                    nc.vector.tensor_copy(zrow[:, D:D + ncols], pb[:, 0:ncols])
                nc.vector.tensor_copy(
                    ed_store[:, nt, :],
                    pb[:, ncols:2 * ncols])
                nc.sync.dma_start(zloc[nt * P:(nt + 1) * P, :], zrow[:])

            def ag_chunk(ch, zloc, ztab):
                # Chunk-major table layout: each AG output is contiguous.
                r0, r1 = CHB[ch], CHB[ch + 1]
                nc.gpsimd.collective_compute(
                    "AllGather", mybir.AluOpType.bypass,
                    replica_groups=[list(range(NCORES))],
                    ins=[zloc[r0:r1, :]],
                    outs=[ztab[NCORES * r0:NCORES * r1, :]])

            xTv = [xTb[c][:] for c in range(4)]
            for nt in range(NT):
                z_tile(nt, xTv, w1b, va1, zloc1, ed1, H, fp8=False)
                if nt in AG_AT:
                    ag_chunk(AG_AT[nt], zloc1, ztab1)

            # ---------- edge phase (shared for both layers) ----------
            gq = [0]                       # rotating SWDGE queue counter
            nidx_reg = nc.gpsimd.to_reg(P * KG)

            def edge_phase1(ztab, idx, ed_store, out_cb):
                # layer 1: bf16 rows, dup'd-pair DVE attention multiply (2x)
                nheads, rep = H, DO
                idx_off = 0
                for nt in range(NT):
                    Kj = KT[nt]
                    nkg = Kj // KG
                    po = pepool.tile([P, D], F32, tag="pout")
                    den = wpool.tile([P, nheads], F32, tag="den")
                    for kg in range(nkg):
                        g = gpool.tile([P, KG, ROWW], BF16, tag="G")
                        nidx = P * KG
                        nc.gpsimd.dma_gather(
                            g[:], ztab[:], idx[:, idx_off:idx_off + nidx // 16],
                            nidx, nidx_reg, ROWW, queue_num=gq[0] % 4)
                        gq[0] += 1
                        idx_off += nidx // 16
                        # attention logits: s = es_gather + ed_local (dup'd pairs)
                        sd = wpool.tile([P, KG, nheads, 2], F32, tag="sd")
                        es_v = g[:, :, D:D + nheads].unsqueeze(3) \
                            .broadcast_to([P, KG, nheads, 2])
                        ed_v = ed_store[:, nt, :].unsqueeze(1).unsqueeze(3) \
                            .broadcast_to([P, KG, nheads, 2])
                        nc.vector.tensor_tensor(sd[:], es_v, ed_v,
                                                mybir.AluOpType.add)
                        # leaky_relu on ScalarE (Prelu shares Exp's table)
                        lk = wpool.tile([P, KG, nheads, 2], F32, tag="lk")
                        nc.scalar.activation(lk[:], sd[:],
                                             mybir.ActivationFunctionType.Prelu,
                                             alpha=ALPHA)
                        ad = wpool.tile([P, KG, nheads, 2], BF16, tag="ad")
                        nc.scalar.activation(ad[:], lk[:],
                                             mybir.ActivationFunctionType.Exp)
                        dpart = wpool.tile([P, nheads], F32, tag="dpart")
                        nc.vector.tensor_reduce(
                            dpart[:], ad[:].rearrange("p k h t -> p h k t"),
                            mybir.AxisListType.XY, mybir.AluOpType.add)
                        if kg == 0:
                            nc.vector.tensor_copy(den[:], dpart[:])
                        else:
                            nc.vector.tensor_tensor(den[:], den[:], dpart[:],
                                                    mybir.AluOpType.add)
                        # single fused attention multiply for all KG slots
                        wg = wgpool.tile([P, KG, D], BF16, tag="wg")
                        g_v = g[:, :, 0:D].rearrange(
                            "p k (h r t) -> p k h r t",
                            h=nheads, r=rep // 2, t=2)
                        a_v = ad[:].unsqueeze(3) \
                            .broadcast_to([P, KG, nheads, rep // 2, 2])
                        w_v = wg[:].rearrange(
                            "p k (h r t) -> p k h r t",
                            h=nheads, r=rep // 2, t=2)
                        nc.vector.tensor_tensor(w_v, g_v, a_v,
                                                mybir.AluOpType.mult)
                        for k in range(KG):
                            kk = kg * KG + k
                            nc.tensor.matmul(po[:], identb[:], wg[:, k, :],
                                             start=(kk == 0), stop=(kk == Kj - 1))
                    # normalize (x2 compensates the dup'd den) and activation
                    rcp = wpool.tile([P, nheads], F32, tag="rcp")
                    nc.vector.reciprocal(rcp[:], den[:])
                    t1 = wpool.tile([P, D], F32, tag="t1")
                    r_v = rcp[:].unsqueeze(2).broadcast_to([P, nheads, rep])
                    t_v = t1[:].rearrange("p (h r) -> p h r", h=nheads, r=rep)
                    nc.vector.scalar_tensor_tensor(
                        t_v, po[:].rearrange("p (h r) -> p h r", h=nheads, r=rep),
                        2.0, r_v, mybir.AluOpType.mult, mybir.AluOpType.mult)
                    out_cb(nt, t1)

            def edge_phase2(ztab, idx, ed_store, out_cb):
                # layer 2: fp8 rows; attention applied via diag(att) fp8
                # matmuls (no per-edge DVE multiply)
                idx_off = 0
                for nt in range(NT):
                    Kj = KT[nt]
                    nkg = Kj // KG
                    po = pepool.tile([P, D], F32, tag="pout")
                    den = wpool.tile([P, 1], F32, tag="den2")
                    for kg in range(nkg):
                        g = gpool.tile([P, KG, ROWB], FP8, tag="G")
                        nidx = P * KG
                        nc.gpsimd.dma_gather(
                            g[:], ztab[:], idx[:, idx_off:idx_off + nidx // 16],
                            nidx, nidx_reg, ROWB, queue_num=gq[0] % 4)
                        gq[0] += 1
                        idx_off += nidx // 16
                        sd = wpool.tile([P, KG], F32, tag="sd2")
                        es_v = g[:, :, D:D + 2].bitcast(BF16) \
                            .rearrange("p k o -> p (k o)")
                        ed_v = ed_store[:, nt, :].broadcast_to([P, KG])
                        nc.vector.tensor_tensor(sd[:], es_v, ed_v,
                                                mybir.AluOpType.add)
                        lk = wpool.tile([P, KG], F32, tag="lk2")
                        nc.scalar.activation(lk[:], sd[:],
                                             mybir.ActivationFunctionType.Prelu,
                                             alpha=ALPHA)
                        ad = wpool.tile([P, KG], BF16, tag="ad2")
                        dpart = wpool.tile([P, 1], F32, tag="dpart2")
                        nc.scalar.activation(ad[:], lk[:],
                                             mybir.ActivationFunctionType.Exp,
                                             accum_out=dpart[:])
                        if kg == 0:
                            nc.vector.tensor_copy(den[:], dpart[:])
                        else:
                            nc.vector.tensor_tensor(den[:], den[:], dpart[:],
                                                    mybir.AluOpType.add)
                        # DG[p, k, j] = ident[p, j] * att[p, k]
                        DG = wgpool.tile([P, KG, P], FP8, tag="DG")
                        i_v = identq[:].unsqueeze(1).broadcast_to([P, KG, P])
                        a_v = ad[:].unsqueeze(2).broadcast_to([P, KG, P])
                        nc.vector.tensor_tensor(DG[:], i_v, a_v,
                                                mybir.AluOpType.mult)
                        for k in range(0, KG, 2):
                            kk = kg * KG + k
                            nc.tensor.matmul(
                                po[:], DG[:, k:k + 2, :], g[:, k:k + 2, 0:D],
                                start=(kk == 0), stop=(kk == Kj - 2),
                                perf_mode=mybir.MatmulPerfMode.DoubleRow)
                    rcp = wpool.tile([P, 1], F32, tag="rcp2")
                    nc.vector.reciprocal(rcp[:], den[:])
                    t1 = wpool.tile([P, D], F32, tag="t1")
                    nc.vector.tensor_tensor(t1[:], po[:],
                                            rcp[:].broadcast_to([P, D]),
                                            mybir.AluOpType.mult)
                    out_cb(nt, t1)

            # layer-1 per-tile epilogue: elu -> bf16 -> transpose into the
            # (dead after z1) xTb tiles, then the z2 tile (hidden under edge1)
            hcTb = xTb
            hcTv = [hcTb[c][:] for c in range(4)]

            def l1_out(nt, t1):
                # elu via ScalarE: em=relu(-t1); ex=exp(-em); pos=relu(t1)
                # elu(x) = max(x, exp(-relu(-x)) - 1)
                em = wpool.tile([P, D], F32, tag="em")
                nc.scalar.activation(em[:], t1[:],
                                     mybir.ActivationFunctionType.Relu,
                                     scale=-1.0)
                ex = wpool.tile([P, D], F32, tag="ex")
                nc.scalar.activation(ex[:], em[:],
                                     mybir.ActivationFunctionType.Exp,
                                     scale=-1.0)
                hc = wpool.tile([P, D], BF16, tag="hc")
                nc.vector.scalar_tensor_tensor(
                    hc[:], ex[:], -1.0, t1[:],
                    mybir.AluOpType.add, mybir.AluOpType.max)
                for cidx in range(4):
                    pt = ptpool.tile([P, P], BF16, tag="ptr")
                    nc.tensor.transpose(pt[:], hc[:, cidx * P:(cidx + 1) * P],
                                        identb[:])
                    nc.vector.tensor_copy(
                        hcTb[cidx][:, nt * P:(nt + 1) * P], pt[:])
                z_tile(nt, hcTv, w2b, va2, zloc2, ed2, 1, fp8=True)
                if nt in AG_AT:
                    # big AG2 chunks early, small last chunk: minimizes the
                    # post-edge1 tail (last chunk depends on z2 tile 15)
                    ag_chunk(AG_AT[nt], zloc2, ztab2)

            edge_phase1(ztab1, idx1, ed1, l1_out)

            # ---------- phase 4: L2 edge + residual ----------
            def l2_out(nt, t1):
                em = wpool.tile([P, D], F32, tag="em")
                nc.scalar.activation(em[:], t1[:],
                                     mybir.ActivationFunctionType.Relu,
                                     scale=-1.0)
                ex = wpool.tile([P, D], F32, tag="ex")
                nc.scalar.activation(ex[:], em[:],
                                     mybir.ActivationFunctionType.Exp,
                                     scale=-1.0)
                el = wpool.tile([P, D], F32, tag="el")
                nc.vector.scalar_tensor_tensor(
                    el[:], ex[:], -1.0, t1[:],
                    mybir.AluOpType.add, mybir.AluOpType.max)
                xr = wpool.tile([P, D], F32, tag="xr")
                nc.sync.dma_start(xr[:], x_in[nt * P:(nt + 1) * P, :])
                ot = wpool.tile([P, D], F32, tag="ot")
                nc.vector.tensor_tensor(ot[:], el[:], xr[:],
                                        mybir.AluOpType.add)
                nc.sync.dma_start(out[nt * P:(nt + 1) * P, :], ot[:])

            edge_phase2(ztab2, idx2, ed2, l2_out)

    nc.compile()
    return nc


def kernel(h, W1, a1, Wout, aout, src, dst):
    h = np.asarray(h, np.float32)
    W1 = np.asarray(W1, np.float32)
    a1 = np.asarray(a1, np.float32)
    Wout = np.asarray(Wout, np.float32)
    aout = np.asarray(aout, np.float32)
    src = np.asarray(src, np.int32)
    dst = np.asarray(dst, np.int32)

    x = h.reshape(N, D)
    nodes, K_sched, totK, idx_cores = _build_host(src, dst)

    key = (tuple(int(k) for k in K_sched), totK)
    if key not in _cache:
        _cache[key] = _build_program(K_sched, totK)
    nc = _cache[key]

    # weight layouts
    W1cat = np.ascontiguousarray(W1.transpose(1, 0, 2).reshape(D, D))
    A1 = np.zeros((D, 16), np.float32)
    for hh in range(H):
        A1[hh * DO:(hh + 1) * DO, hh] = a1[hh, :DO]
        A1[hh * DO:(hh + 1) * DO, 8 + hh] = a1[hh, DO:]
    A2 = np.stack([aout[:D], aout[D:]], axis=1).astype(np.float32)
    bf16 = ml_dtypes.bfloat16
    VA1 = np.ascontiguousarray(W1cat @ A1).astype(bf16)
    VA2 = np.ascontiguousarray(Wout @ A2).astype(bf16)
    W1b = W1cat.astype(bf16)
    W2b = Wout.astype(bf16)
    ident = np.eye(P, dtype=np.float32).astype(bf16)

    in_maps = []
    for c in range(NCORES):
        xs = np.ascontiguousarray(x[nodes[c]])
        in_maps.append({
            "xT": np.ascontiguousarray(xs.T).astype(bf16),
            "x": xs,
            "w1": W1b,
            "w2": W2b,
            "va1": VA1,
            "va2": VA2,
            "ident": ident,
            "idx1": idx_cores[c],
        })

    trace = bool(int(os.environ.get("GAT_TRACE", "0")))
    res = run_bass_kernel_spmd(nc, in_maps, core_ids=list(range(NCORES)),
                               trace=trace)
    if trace:
        print("HW exec time:", res.exec_time_ns, "ns")
        print("trace:", res.instructions_and_trace[1]
              if res.instructions_and_trace else None)
    outf = np.zeros((N, D), np.float32)
    for c in range(NCORES):
        outf[nodes[c]] = res.results[c]["out"]
    return outf.reshape(B, S, D)

